# revision 1
# baseline (speedup 1.0000x reference)
"""Binarized 3x3 conv (stride 1, pad 1) + bias on 8 Trainium2 NeuronCores.

Full problem: x[32,256,56,56] f32, weight[256,256,3,3] f32, bias[256] f32
-> y[32,256,56,56] f32 with y = conv2d(sign(x), sign(weight), pad=1) + bias
(sign(t) = +1 for t >= 0 else -1).

Sharding: data-parallel over batch. Each of the 8 cores gets 4 images and a
replicated copy of weight/bias, computes its shard fully on-device, and the
host concatenates the 8 output shards.

Per-core kernel:
  - binarize x and w to +/-0.5 with one fused DVE op each ((v>=0) - 0.5);
    the final PSUM->SBUF copy applies scale=4 to undo the 0.25 product
    scale, so results are exactly the +/-1 conv (all integers, exact in f32).
  - x lives zero-padded in SBUF as [128(ci_p), 2(ci_blk), 3376] bf16 per
    image: 58x58 padded image rows + 1 guard element front/back.
  - weight is transposed on the PE (36 x 128x128 transposes via identity)
    into lhsT [128(ci_p), 2(ci_blk), 9(tap), 256(co)] bf16.
  - conv: for each (co_blk, image, 8-row output chunk): accumulate
    18 matmuls (9 taps x 2 ci_blk), K=128/M=128/N=464, into one PSUM bank.
    Outputs computed at the 2 pad columns of each row are garbage and are
    skipped when copying out.
  - PSUM -> SBUF via ScalarE: Identity(psum*4 + bias[co]), then DMA to y.
"""

import numpy as np

import concourse.bacc as bacc
import concourse.mybir as mybir
import concourse.tile as tile
from concourse.bass_utils import run_bass_kernel_spmd
from concourse.masks import make_identity

F32 = mybir.dt.float32
AF = mybir.ActivationFunctionType
ALU = mybir.AluOpType

N_CORES = 8
N_IMGS = 4         # images per core (32 / 8)
H = W = 56
WP = 58            # padded row width
CIN = 256
COUT = 256
CI_BLKS = 2        # 256 ci = 2 x 128 partitions
CO_BLKS = 2
R = 8              # output rows per chunk
NCHUNK = H // R    # 7
NV = R * WP        # 464 matmul moving free size
IMG_FA = 3376      # aligned per-ci_blk padded image elems (58*58+2 -> 3376)

BIN_DT = mybir.dt.bfloat16


def _build_conv(tc, y_ap, x_ap, w_ap, b_ap):
    nc = tc.nc
    scale = 4.0  # undo (+/-0.5)*(+/-0.5) = +/-0.25 product scale

    with (
        tc.tile_pool(name="consts", bufs=1) as consts,
        tc.tile_pool(name="wstage", bufs=1) as wstage_pool,
        tc.tile_pool(name="lhst", bufs=1) as lhst_pool,
        tc.tile_pool(name="xstage", bufs=2) as xstage_pool,
        tc.tile_pool(name="xpad", bufs=2) as xpad_pool,
        tc.tile_pool(name="outsb", bufs=4) as out_pool,
        tc.tile_pool(name="tpsum", bufs=4, space="PSUM") as tpsum_pool,
        tc.tile_pool(name="cpsum", bufs=4, space="PSUM") as cpsum_pool,
    ):
        # --- constants -----------------------------------------------------
        ident = consts.tile([128, 128], BIN_DT)
        make_identity(nc, ident)
        bias_sb = consts.tile([128, CO_BLKS], F32)
        nc.sync.dma_start(out=bias_sb, in_=b_ap.rearrange("(b p) -> p b", p=128))

        # --- weight prep ---------------------------------------------------
        wstage = wstage_pool.tile([128, CO_BLKS, CIN, 9], F32)
        nc.sync.dma_start(
            out=wstage,
            in_=w_ap.rearrange("(b p) ci kh kw -> p b ci (kh kw)", p=128),
        )
        wb = wstage_pool.tile([128, CO_BLKS, CIN, 9], BIN_DT)
        nc.vector.tensor_scalar(wb, wstage, 0.0, 0.5, ALU.is_ge, ALU.subtract)

        # lhsT[ci_p, ci_blk, tap, co]
        lhst = lhst_pool.tile([128, CI_BLKS, 9, COUT], BIN_DT)
        for c in range(CO_BLKS):
            for b in range(CI_BLKS):
                for t in range(9):
                    tp = tpsum_pool.tile([128, 128], BIN_DT)
                    nc.tensor.transpose(tp, wb[:, c, b * 128:(b + 1) * 128, t], ident)
                    nc.vector.tensor_copy(out=lhst[:, b, t, c * 128:(c + 1) * 128],
                                          in_=tp)

        # --- per-image pipeline -------------------------------------------
        for n in range(N_IMGS):
            xstage = xstage_pool.tile([128, CI_BLKS, H * W], F32)
            nc.sync.dma_start(
                out=xstage,
                in_=x_ap[n].rearrange("(b p) h w -> p b (h w)", p=128),
            )
            xpad = xpad_pool.tile([128, CI_BLKS, IMG_FA], BIN_DT)
            nc.vector.memset(xpad, 0.0)
            for b in range(CI_BLKS):
                # data rows: padded row h+1, cols 1..56; base elem 1+(h+1)*58+1
                dst = xpad[:, b, 60:60 + H * WP].rearrange(
                    "p (h w) -> p h w", w=WP)[:, :, 0:W]
                src = xstage[:, b].rearrange("p (h w) -> p h w", w=W)
                nc.vector.tensor_scalar(dst, src, 0.0, 0.5, ALU.is_ge, ALU.subtract)

            for c in range(CO_BLKS):
                for k in range(NCHUNK):
                    ps = cpsum_pool.tile([128, NV], F32)
                    mm = 0
                    for b in range(CI_BLKS):
                        for t in range(9):
                            kh, kw = divmod(t, 3)
                            base = (R * k + kh) * WP + kw  # incl. -1 guard shift
                            nc.tensor.matmul(
                                ps,
                                lhst[:, b, t, c * 128:(c + 1) * 128],
                                xpad[:, b, base:base + NV],
                                start=(mm == 0),
                                stop=(mm == 17),
                            )
                            mm += 1
                    osb = out_pool.tile([128, R * W], F32)
                    nc.scalar.activation(
                        out=osb.rearrange("p (r w) -> p r w", w=W),
                        in_=ps.rearrange("p (r w) -> p r w", w=WP)[:, :, 1:57],
                        func=AF.Identity,
                        bias=bias_sb[:, c:c + 1],
                        scale=scale,
                    )
                    nc.sync.dma_start(
                        out=y_ap[n, c * 128:(c + 1) * 128]
                            .rearrange("co h w -> co (h w)")[:, R * W * k:R * W * (k + 1)],
                        in_=osb,
                    )


_NC_CACHE = None


def _get_nc():
    global _NC_CACHE
    if _NC_CACHE is None:
        nc = bacc.Bacc("TRN2", target_bir_lowering=False, debug=False)
        x_ap = nc.dram_tensor("x", [N_IMGS, CIN, H, W], F32,
                              kind="ExternalInput").ap()
        w_ap = nc.dram_tensor("weight", [COUT, CIN, 3, 3], F32,
                              kind="ExternalInput").ap()
        b_ap = nc.dram_tensor("bias", [COUT], F32, kind="ExternalInput").ap()
        y_ap = nc.dram_tensor("y", [N_IMGS, COUT, H, W], F32,
                              kind="ExternalOutput").ap()
        with tile.TileContext(nc) as tc:
            _build_conv(tc, y_ap, x_ap, w_ap, b_ap)
        nc.compile()
        _NC_CACHE = nc
    return _NC_CACHE


def kernel(x: np.ndarray, weight: np.ndarray, bias: np.ndarray) -> np.ndarray:
    assert x.shape == (32, CIN, H, W), x.shape
    x = np.ascontiguousarray(x, dtype=np.float32)
    weight = np.ascontiguousarray(weight, dtype=np.float32)
    bias = np.ascontiguousarray(bias, dtype=np.float32)

    nc = _get_nc()
    shards = [x[i * N_IMGS:(i + 1) * N_IMGS] for i in range(N_CORES)]
    in_maps = [{"x": s, "weight": weight, "bias": bias} for s in shards]
    res = run_bass_kernel_spmd(nc, in_maps, core_ids=list(range(N_CORES)))
    return np.concatenate([r["y"] for r in res.results], axis=0)


# revision 21
# speedup vs baseline: 46692.9048x; 46692.9048x over previous
"""Binarized 3x3 conv (stride 1, pad 1) + bias on 8 Trainium2 NeuronCores.

Full problem: x[32,256,56,56] f32, weight[256,256,3,3] f32, bias[256] f32
-> y[32,256,56,56] f32 with y = conv2d(sign(x), sign(weight), pad=1) + bias
(sign(t) = +1 for t >= 0 else -1).

Sharding: data-parallel over batch. Each of the 8 cores gets 4 images and a
replicated copy of weight/bias, computes its shard fully on-device, and the
host concatenates the 8 output shards.

Per-core kernel:
  - binarize x and w to +/-0.5 with one fused DVE op each ((v>=0) - 0.5);
    the final PSUM->SBUF copy applies scale=4 to undo the 0.25 product
    scale, so results are exactly the +/-1 conv (all integers, exact in f32).
  - x lives zero-padded in SBUF as [128(ci_p), 2(ci_blk), 3376] fp8 per
    image: 58x58 padded image rows + 1 guard element front/back. Pad zeros
    are written once per buffer; data rows are rewritten per image.
  - weight is binarized to bf16, transposed on the PE (36 x 128x128
    transposes via identity), and stored as fp8 lhsT
    [128(ci_p), 2(ci_blk), 9(tap), 256(co)].
  - conv: for each (co_blk, image, 8-row output chunk): accumulate 9
    DoubleRow fp8 matmuls (one per tap, K=256 packed as [128,2]),
    M=128/N=464, into one PSUM bank. Outputs computed at the 2 pad columns
    of each row are garbage and are skipped on the way out.
  - PSUM -> SBUF via ScalarE: Identity(psum*4 + bias[co]), then DMA to y.
"""

import numpy as np

import concourse.bacc as bacc
import concourse.mybir as mybir
import concourse.tile as tile
from concourse.bass_utils import run_bass_kernel_spmd
from concourse.masks import make_identity

F32 = mybir.dt.float32
BF16 = mybir.dt.bfloat16
FP8 = mybir.dt.float8e4
AF = mybir.ActivationFunctionType
ALU = mybir.AluOpType
DR = mybir.MatmulPerfMode.DoubleRow

N_CORES = 8
H = W = 56
WP = 58            # padded row width
CIN = 256
COUT = 256
CI_BLKS = 2        # 256 ci = 2 x 128 partitions
CO_BLKS = 2
R = 8              # output rows per chunk
NCHUNK = H // R    # 7
NV = R * WP        # 464 matmul moving free size
IMG_FA = 3376      # aligned per-ci_blk padded image elems (58*58+2 -> 3376)


def _build_conv(tc, y_ap, x_ap, w_ap, b_ap, n_imgs):
    nc = tc.nc
    scale = 4.0  # undo (+/-0.5)*(+/-0.5) = +/-0.25 product scale

    with (
        tc.tile_pool(name="consts", bufs=1) as consts,
        tc.tile_pool(name="wstage", bufs=1) as wstage_pool,
        tc.tile_pool(name="lhst", bufs=1) as lhst_pool,
        tc.tile_pool(name="xstage", bufs=2) as xstage_pool,
        tc.tile_pool(name="xpad", bufs=1) as xpad_pool,
        tc.tile_pool(name="outsb", bufs=4) as out_pool,
        tc.tile_pool(name="tpsum", bufs=2, space="PSUM") as tpsum_pool,
        tc.tile_pool(name="cpsum", bufs=6, space="PSUM") as cpsum_pool,
    ):
        # --- constants -----------------------------------------------------
        ident = consts.tile([128, 128], BF16)
        make_identity(nc, ident)
        junk = consts.tile([128, 512], BF16, name="junk")
        nc.gpsimd.memset(junk, 0.0)

        # --- DMA issue order is bandwidth-critical: the conv stream can't
        # start until W_c0 + the first x rows are in SBUF (~360 GB/s/core).
        wstage = wstage_pool.tile([128, CO_BLKS, CIN, 9], F32)
        wb = wstage_pool.tile([128, CO_BLKS, CIN, 9], BF16)
        lhst = lhst_pool.tile([128, CI_BLKS, 9, COUT], FP8)
        xstage0 = xstage_pool.tile([128, CI_BLKS, H * W], F32,
                                   name="xstage0", tag="xstage")

        def dma_w(c, b):
            # one quarter of the weights: co block c, ci block b
            nc.sync.dma_start(
                out=wstage[:, c, b * 128:(b + 1) * 128],
                in_=w_ap[c * 128:(c + 1) * 128, b * 128:(b + 1) * 128].rearrange(
                    "co ci kh kw -> co ci (kh kw)"),
            )

        def dma_x(xstage, n, r0, r1, b):
            nc.sync.dma_start(
                out=xstage[:, b, r0 * W:r1 * W],
                in_=x_ap[n, b * 128:(b + 1) * 128, r0:r1]
                    .rearrange("c h w -> c (h w)"),
            )

        # interleave so the bytes gating the first conv chunk arrive first:
        # lhsT(c=0) needs both W_c0 quarters; chunk k=0 needs x rows 0-27
        dma_w(0, 0)
        dma_w(0, 1)
        dma_x(xstage0, 0, 0, 28, 0)
        dma_x(xstage0, 0, 0, 28, 1)
        dma_w(1, 0)
        dma_w(1, 1)
        dma_x(xstage0, 0, 28, H, 0)
        dma_x(xstage0, 0, 28, H, 1)
        bias_sb = consts.tile([128, CO_BLKS], F32)
        nc.scalar.dma_start(out=bias_sb, in_=b_ap.rearrange("(b p) -> p b", p=128))

        # --- weight prep (PSUM->SBUF casts on ScalarE); DVE work emitted in
        # DMA-arrival order ---------------------------------------------------
        # lhsT[ci_p, ci_blk, tap, co] in fp8 (cast on the PSUM->SBUF copy)
        def binz(dst, src):
            nc.vector.tensor_scalar(dst, src, 0.0, 0.5, ALU.is_ge, ALU.subtract)

        def wprep(c, b):
            binz(wb[:, c, b * 128:(b + 1) * 128], wstage[:, c, b * 128:(b + 1) * 128])
            for t in range(9):
                tp = tpsum_pool.tile([128, 128], BF16)
                nc.tensor.transpose(tp, wb[:, c, b * 128:(b + 1) * 128, t], ident)
                nc.scalar.copy(out=lhst[:, b, t, c * 128:(c + 1) * 128],
                               in_=tp)

        # --- x buffers: persistent padded buffers, pad zeros written once
        NXPAD = 3
        xpads = [xpad_pool.tile([128, CI_BLKS, IMG_FA], FP8,
                                name=f"xpad{i}", tag=f"xpad{i}")
                 for i in range(NXPAD)]
        for xp in xpads:
            for b in range(CI_BLKS):
                # head guard + top pad row (+ first in-row pad col): elems 0..59
                nc.vector.memset(xp[:, b, 0:60], 0.0)
                # bottom pad row + tail guard: elems 1+57*58 .. 3375
                nc.vector.memset(xp[:, b, 1 + 57 * WP:IMG_FA], 0.0)
                # per-row right+left pad pairs at (1+h*58+57, 1+h*58+58)
                nc.vector.memset(
                    xp[:, b, 58:58 + 57 * WP].rearrange(
                        "p (h w) -> p h w", w=WP)[:, :, 0:2],
                    0.0,
                )

        # --- per-image pipeline -------------------------------------------
        def binz_x(xstage, xpad, r0, r1, b):
            # data rows: padded row h+1, cols 1..56
            dst = xpad[:, b, 60:60 + H * WP].rearrange(
                "p (h w) -> p h w", w=WP)[:, r0:r1, 0:W]
            src = xstage[:, b].rearrange("p (h w) -> p h w", w=W)[:, r0:r1]
            binz(dst, src)

        def conv_chunk(n, xpad, c, k):
            ps = cpsum_pool.tile([128, NV], F32, name="ps", tag="ps")
            for t in range(9):
                kh, kw = divmod(t, 3)
                base = (R * k + kh) * WP + kw  # incl. -1 guard shift
                nc.tensor.matmul(
                    ps,
                    lhst[:, 0:2, t, c * 128:(c + 1) * 128],
                    xpad[:, 0:2, base:base + NV],
                    start=(t == 0),
                    stop=(t == 8),
                    perf_mode=DR,
                )
            osb = out_pool.tile([128, R * W], F32, name="osb")
            nc.scalar.activation(
                out=osb.rearrange("p (r w) -> p r w", w=W),
                in_=ps.rearrange("p (r w) -> p r w", w=WP)[:, :, 1:57],
                func=AF.Identity,
                bias=bias_sb[:, c:c + 1],
                scale=scale,
            )
            nc.sync.dma_start(
                out=y_ap[n, c * 128:(c + 1) * 128]
                    .rearrange("co h w -> co (h w)")[:, R * W * k:R * W * (k + 1)],
                in_=osb,
            )

        # HAM warm-up: ~12 throwaway matmuls on zeros keep the PE activity
        # monitor busy so the real stream starts un-throttled (2.4 GHz)
        for _ in range(12):
            jps = cpsum_pool.tile([128, 512], F32, name="ps", tag="ps")
            nc.tensor.matmul(jps, junk[:, :128], junk, start=True, stop=True)

        for n in range(n_imgs):
            xpad = xpads[n % NXPAD]
            if n == 0:
                # emission order = engine program order (DMA-arrival order)
                wprep(0, 0)
                wprep(0, 1)
                binz_x(xstage0, xpad, 0, 28, 0)
                binz_x(xstage0, xpad, 0, 28, 1)
                wprep(1, 0)
                wprep(1, 1)
                binz_x(xstage0, xpad, 28, H, 0)
                binz_x(xstage0, xpad, 28, H, 1)
                for c in range(CO_BLKS):
                    for k in range(NCHUNK):
                        conv_chunk(0, xpad, c, k)
            else:
                xstage = xstage_pool.tile([128, CI_BLKS, H * W], F32,
                                          name=f"xstage{n}", tag="xstage")
                for r0, r1 in ((0, 28), (28, H)):
                    for b in range(CI_BLKS):
                        dma_x(xstage, n, r0, r1, b)
                        binz_x(xstage, xpad, r0, r1, b)
                for c in range(CO_BLKS):
                    for k in range(NCHUNK):
                        conv_chunk(n, xpad, c, k)


_NC_CACHE = {}


def _get_nc(n_imgs):
    if n_imgs not in _NC_CACHE:
        nc = bacc.Bacc("TRN2", target_bir_lowering=False, debug=False)
        x_ap = nc.dram_tensor("x", [n_imgs, CIN, H, W], F32,
                              kind="ExternalInput").ap()
        w_ap = nc.dram_tensor("weight", [COUT, CIN, 3, 3], F32,
                              kind="ExternalInput").ap()
        b_ap = nc.dram_tensor("bias", [COUT], F32, kind="ExternalInput").ap()
        y_ap = nc.dram_tensor("y", [n_imgs, COUT, H, W], F32,
                              kind="ExternalOutput").ap()
        with tile.TileContext(nc) as tc:
            _build_conv(tc, y_ap, x_ap, w_ap, b_ap, n_imgs)
        nc.compile()
        _NC_CACHE[n_imgs] = nc
    return _NC_CACHE[n_imgs]


def kernel(x: np.ndarray, weight: np.ndarray, bias: np.ndarray) -> np.ndarray:
    assert x.shape[1:] == (CIN, H, W), x.shape
    assert x.shape[0] % N_CORES == 0, x.shape
    n_imgs = x.shape[0] // N_CORES
    x = np.ascontiguousarray(x, dtype=np.float32)
    weight = np.ascontiguousarray(weight, dtype=np.float32)
    bias = np.ascontiguousarray(bias, dtype=np.float32)

    nc = _get_nc(n_imgs)
    shards = [x[i * n_imgs:(i + 1) * n_imgs] for i in range(N_CORES)]
    in_maps = [{"x": s, "weight": weight, "bias": bias} for s in shards]
    res = run_bass_kernel_spmd(nc, in_maps, core_ids=list(range(N_CORES)))
    return np.concatenate([r["y"] for r in res.results], axis=0)
